# revision 3
# baseline (speedup 1.0000x reference)
"""DGCN (GCNConv + self/change terms) on 8 Trainium2 NeuronCores.

Strategy (dst-sharded graph parallelism):
  - Output nodes (segment-sum destinations) are sharded across the 8 cores;
    each core owns a contiguous range of 64-node "dst tiles".
  - Host sorts edges (incl. self-loops) by (dst tile, src), pads each tile's
    edge list to multiples of 128, and builds per-core tables:
      idx[128, B]  int32  source-row gather indices (partition p, block j)
      dstl[128, B] f32    local dst (0..63) within the tile
      nrm[128, B]  f32    edge weight dinv[src]*dinv[dst] (0 for padding)
  - Device, per dst tile t: indirect-DMA gather of x[src] rows (128 rows per
    block, F_t blocks in one DMA), build a one-hot matrix
    oh[e, dst] = (iota == dstl_e) * nrm_e on the vector engine, and
    accumulate zT[d, dst] += msgs_e^T @ oh on the tensor engine in PSUM.
    This performs the whole normalized scatter-add as matmuls.
  - Algebraic folding: out = h_neigh + x@W0 + (h_neigh - x)@Wt
        = (z @ Wc + bc) @ (I + Wt) + x @ (W0 - Wt)
        = z @ C + x @ B2 + b'
    with C = Wc @ (I + Wt), B2 = W0 - Wt, b' = bc @ (I + Wt), and z the
    normalized neighbor sum (incl. self loops) of raw x rows. So the x@Wc
    matmul is applied *after* aggregation on 64-row tiles (8x less matmul
    work than computing x@Wc for all N on every core) and each core needs
    only two small constant weights.
"""

import numpy as np

N_NODES = 50000
D = 128
N_CORES = 8
TILE_DST = 64  # dst nodes per tile (matmul free dim)
BLK = 128  # edges per matmul block (PE contraction dim)

_NC_CACHE = {}


def _host_prep(x, edge_index, Wc, bc, W0, Wt, n_cores=N_CORES, tile_dst=TILE_DST):
    n, d = x.shape
    src = np.asarray(edge_index[0], dtype=np.int64)
    dst = np.asarray(edge_index[1], dtype=np.int64)

    # in-degree incl. self loop -> symmetric normalization factors
    deg = (np.bincount(dst, minlength=n) + 1).astype(np.float32)
    dinv = (1.0 / np.sqrt(deg)).astype(np.float32)

    loops = np.arange(n, dtype=np.int64)
    src_a = np.concatenate([src, loops])
    dst_a = np.concatenate([dst, loops])
    norm_a = (dinv[src_a] * dinv[dst_a]).astype(np.float32)

    tiles_total = -(-n // tile_dst)
    tiles_total = -(-tiles_total // n_cores) * n_cores
    tpc = tiles_total // n_cores
    n_pad = tiles_total * tile_dst
    rows_pc = tpc * tile_dst

    tile_of = dst_a // tile_dst
    order = np.lexsort((src_a, tile_of))
    src_s = src_a[order].astype(np.int32)
    dstl_s = (dst_a[order] - tile_of[order] * tile_dst).astype(np.float32)
    norm_s = norm_a[order]
    tile_s = tile_of[order]

    counts = np.bincount(tile_s, minlength=tiles_total)
    blocks = -(-counts.reshape(n_cores, tpc) // BLK)
    F = np.maximum(blocks.max(axis=0), 1).astype(np.int64)  # per tile-slot
    B = int(F.sum())
    off = np.zeros(tpc, np.int64)
    off[1:] = np.cumsum(F)[:-1]

    idx_t = np.zeros((n_cores, BLK, B), np.int32)
    dst_t = np.zeros((n_cores, BLK, B), np.float32)
    nrm_t = np.zeros((n_cores, BLK, B), np.float32)
    tile_starts = np.zeros(tiles_total + 1, np.int64)
    tile_starts[1:] = np.cumsum(counts)
    for k in range(n_cores):
        for i in range(tpc):
            g = k * tpc + i
            c = int(counts[g])
            if c == 0:
                continue
            fi = int(F[i])
            cap = fi * BLK
            s0 = int(tile_starts[g])
            bi = np.zeros(cap, np.int32)
            bd = np.zeros(cap, np.float32)
            bn = np.zeros(cap, np.float32)
            bi[:c] = src_s[s0 : s0 + c]
            bd[:c] = dstl_s[s0 : s0 + c]
            bn[:c] = norm_s[s0 : s0 + c]
            cols = slice(int(off[i]), int(off[i]) + fi)
            idx_t[k][:, cols] = bi.reshape(fi, BLK).T
            dst_t[k][:, cols] = bd.reshape(fi, BLK).T
            nrm_t[k][:, cols] = bn.reshape(fi, BLK).T

    # fused weights
    Wc64 = np.asarray(Wc, np.float64)
    Wt64 = np.asarray(Wt, np.float64)
    W064 = np.asarray(W0, np.float64)
    bc64 = np.asarray(bc, np.float64)
    B1 = np.eye(d) + Wt64
    C = (Wc64 @ B1).astype(np.float32)
    B2 = (W064 - Wt64).astype(np.float32)
    bp = (bc64 @ B1).astype(np.float32)

    x_pad = np.zeros((n_pad, d), np.float32)
    x_pad[:n] = np.asarray(x, np.float32)

    consts = {
        "cw": C,
        "b2w": B2,
        "bpb": np.broadcast_to(bp, (tile_dst, d)).copy(),
        "iota": np.broadcast_to(
            np.arange(tile_dst, dtype=np.float32), (BLK, tile_dst)
        ).copy(),
        "ident": np.eye(BLK, dtype=np.float32),
    }
    in_maps = []
    for k in range(n_cores):
        m = dict(consts)
        m["x_full"] = x_pad
        m["x_own"] = x_pad[k * rows_pc : (k + 1) * rows_pc].copy()
        m["idx_t"] = idx_t[k]
        m["dst_t"] = dst_t[k]
        m["nrm_t"] = nrm_t[k]
        in_maps.append(m)

    meta = dict(F=F, off=off, B=B, tpc=tpc, n_pad=n_pad, rows_pc=rows_pc, d=d)
    return in_maps, meta


def _build_nc(meta, n_cores=N_CORES, tile_dst=TILE_DST):
    import concourse.bass as bass
    import concourse.bacc as bacc
    import concourse.mybir as mybir
    import concourse.tile as tile

    f32 = mybir.dt.float32
    F, off = meta["F"], meta["off"]
    B, tpc = meta["B"], meta["tpc"]
    n_pad, rows_pc, d = meta["n_pad"], meta["rows_pc"], meta["d"]

    nc = bacc.Bacc("TRN2", target_bir_lowering=False, debug=False, num_devices=n_cores)
    x_full = nc.declare_dram_parameter("x_full", [n_pad, d], f32, isOutput=False)
    x_own = nc.declare_dram_parameter("x_own", [rows_pc, d], f32, isOutput=False)
    idx_t = nc.declare_dram_parameter("idx_t", [BLK, B], mybir.dt.int32, isOutput=False)
    dst_t = nc.declare_dram_parameter("dst_t", [BLK, B], f32, isOutput=False)
    nrm_t = nc.declare_dram_parameter("nrm_t", [BLK, B], f32, isOutput=False)
    cw = nc.declare_dram_parameter("cw", [d, d], f32, isOutput=False)
    b2w = nc.declare_dram_parameter("b2w", [d, d], f32, isOutput=False)
    bpb = nc.declare_dram_parameter("bpb", [tile_dst, d], f32, isOutput=False)
    iota = nc.declare_dram_parameter("iota", [BLK, tile_dst], f32, isOutput=False)
    ident = nc.declare_dram_parameter("ident", [BLK, BLK], f32, isOutput=False)
    out = nc.declare_dram_parameter("out", [rows_pc, d], f32, isOutput=True)

    eq, mul, add = (
        mybir.AluOpType.is_equal,
        mybir.AluOpType.mult,
        mybir.AluOpType.add,
    )

    with tile.TileContext(nc) as tc:
        with (
            tc.tile_pool(name="const", bufs=1) as cpool,
            tc.tile_pool(name="tbl", bufs=1) as tpool,
            tc.tile_pool(name="gather", bufs=3) as gpool,
            tc.tile_pool(name="work", bufs=3) as wpool,
            tc.tile_pool(name="oh", bufs=4) as ohpool,
            tc.tile_pool(name="zps", bufs=2, space="PSUM") as zpool,
            tc.tile_pool(name="tps", bufs=2, space="PSUM") as tpspool,
            tc.tile_pool(name="ops", bufs=2, space="PSUM") as opool,
        ):
            c_sb = cpool.tile([d, d], f32)
            nc.sync.dma_start(out=c_sb[:], in_=cw[:])
            b2_sb = cpool.tile([d, d], f32)
            nc.sync.dma_start(out=b2_sb[:], in_=b2w[:])
            bp_sb = cpool.tile([tile_dst, d], f32)
            nc.sync.dma_start(out=bp_sb[:], in_=bpb[:])
            io_sb = cpool.tile([BLK, tile_dst], f32)
            nc.sync.dma_start(out=io_sb[:], in_=iota[:])
            id_sb = cpool.tile([BLK, BLK], f32)
            nc.sync.dma_start(out=id_sb[:], in_=ident[:])
            ix_sb = tpool.tile([BLK, B], mybir.dt.int32)
            nc.sync.dma_start(out=ix_sb[:], in_=idx_t[:])
            dl_sb = tpool.tile([BLK, B], f32)
            nc.sync.dma_start(out=dl_sb[:], in_=dst_t[:])
            nm_sb = tpool.tile([BLK, B], f32)
            nc.sync.dma_start(out=nm_sb[:], in_=nrm_t[:])

            for i in range(tpc):
                fi = int(F[i])
                o = int(off[i])
                g = gpool.tile([BLK, fi * d], f32, tag="g")
                for j in range(fi):
                    # HW indirect DMA supports one gathered row per partition
                    # per instruction (idx [128,1] -> out [128,128]).
                    nc.gpsimd.indirect_dma_start(
                        out=g[:, j * d : (j + 1) * d],
                        out_offset=None,
                        in_=x_full[:],
                        in_offset=bass.IndirectOffsetOnAxis(
                            ap=ix_sb[:, o + j : o + j + 1], axis=0
                        ),
                    )
                xo = wpool.tile([tile_dst, d], f32, tag="xo")
                nc.sync.dma_start(
                    out=xo[:], in_=x_own[i * tile_dst : (i + 1) * tile_dst, :]
                )
                xt_ps = tpspool.tile([d, tile_dst], f32)
                nc.tensor.transpose(
                    out=xt_ps[:], in_=xo[:], identity=id_sb[:tile_dst, :tile_dst]
                )
                xt_sb = wpool.tile([d, tile_dst], f32, tag="xt")
                nc.scalar.copy(out=xt_sb[:], in_=xt_ps[:])

                z_ps = zpool.tile([d, tile_dst], f32)
                for j in range(fi):
                    oh = ohpool.tile([BLK, tile_dst], f32, tag="oh")
                    nc.vector.tensor_scalar(
                        out=oh[:],
                        in0=io_sb[:],
                        scalar1=dl_sb[:, o + j : o + j + 1],
                        scalar2=nm_sb[:, o + j : o + j + 1],
                        op0=eq,
                        op1=mul,
                    )
                    nc.tensor.matmul(
                        out=z_ps[:],
                        lhsT=g[:, j * d : (j + 1) * d],
                        rhs=oh[:],
                        start=(j == 0),
                        stop=(j == fi - 1),
                    )
                z_sb = wpool.tile([d, tile_dst], f32, tag="z")
                nc.scalar.copy(out=z_sb[:], in_=z_ps[:])

                o_ps = opool.tile([tile_dst, d], f32)
                nc.tensor.matmul(
                    out=o_ps[:], lhsT=z_sb[:], rhs=c_sb[:], start=True, stop=False
                )
                nc.tensor.matmul(
                    out=o_ps[:], lhsT=xt_sb[:], rhs=b2_sb[:], start=False, stop=True
                )
                o_sb = wpool.tile([tile_dst, d], f32, tag="o")
                nc.vector.tensor_tensor(out=o_sb[:], in0=o_ps[:], in1=bp_sb[:], op=add)
                nc.sync.dma_start(
                    out=out[i * tile_dst : (i + 1) * tile_dst, :], in_=o_sb[:]
                )
    nc.compile()
    return nc


def _get_nc(meta, n_cores=N_CORES, tile_dst=TILE_DST):
    key = (tuple(int(f) for f in meta["F"]), meta["n_pad"], n_cores, tile_dst)
    if key not in _NC_CACHE:
        _NC_CACHE[key] = _build_nc(meta, n_cores=n_cores, tile_dst=tile_dst)
    return _NC_CACHE[key]


_LAST_RESULTS = None


def kernel(x, edge_index, Wc, bc, W0, Wt):
    global _LAST_RESULTS
    from concourse.bass_utils import run_bass_kernel_spmd

    x = np.asarray(x)
    n = x.shape[0]
    in_maps, meta = _host_prep(x, edge_index, Wc, bc, W0, Wt)
    nc = _get_nc(meta)
    res = run_bass_kernel_spmd(nc, in_maps, list(range(N_CORES)))
    _LAST_RESULTS = res
    outs = [res.results[k]["out"] for k in range(N_CORES)]
    return np.concatenate(outs, axis=0)[:n].astype(np.float32)


# revision 11
# speedup vs baseline: 2.6111x; 2.6111x over previous
"""DGCN (GCNConv + self/change terms) on 8 Trainium2 NeuronCores.

Strategy (dst-sharded graph parallelism):
  - Output nodes (segment-sum destinations) are sharded across the 8 cores;
    each core owns a contiguous range of 64-node "dst tiles".
  - Host sorts edges (incl. self-loops) by (dst tile, src), pads each tile's
    edge list to multiples of 128, and builds per-core tables:
      ix16[128, 8*B] int16 gather indices (dma_gather layout: flat edge i of
                          a call at [i%16, i//16], replicated to the 8
                          16-partition Q7 groups)
      dstl[128, B] f32    local dst (0..63) within the tile
      nrm[128, B]  f32    edge weight dinv[src]*dinv[dst] (0 for padding)
  - Device, per dst tile t: dma_gather of x[src] rows (up to 512 rows per
    call), build a one-hot matrix oh[e, dst] = (iota == dstl_e) * nrm_e on
    the vector engine, and accumulate zT[d, dst] += msgs_e^T @ oh on the
    tensor engine in PSUM. This performs the whole normalized scatter-add
    as matmuls.
  - dma_gather indices are int16, so the gather table is split in two DRAM
    tensors: x_full rows [0, 32768) and x_hi rows [32768, n_pad); each
    tile's (src-sorted) edges are split lo/hi at block granularity.
  - Algebraic folding: out = h_neigh + x@W0 + (h_neigh - x)@Wt
        = (z @ Wc + bc) @ (I + Wt) + x @ (W0 - Wt)
        = z @ C + x @ B2 + b'
    with C = Wc @ (I + Wt), B2 = W0 - Wt, b' = bc @ (I + Wt), and z the
    normalized neighbor sum (incl. self loops) of raw x rows. So the x@Wc
    matmul is applied *after* aggregation on 64-row tiles (8x less matmul
    work than computing x@Wc for all N on every core) and each core needs
    only two small constant weights.
"""

import numpy as np

N_NODES = 50000
D = 128
N_CORES = 8
TILE_DST = 64  # dst nodes per tile (matmul free dim)
BLK = 128  # edges per matmul block (PE contraction dim)
HALF = 32768  # int16 index limit -> gather table split point
CALL_BLKS = 4  # max blocks (128 idxs each) per dma_gather call

_NC_CACHE = {}


def _host_prep(x, edge_index, Wc, bc, W0, Wt, n_cores=N_CORES, tile_dst=TILE_DST):
    n, d = x.shape
    src = np.asarray(edge_index[0], dtype=np.int64)
    dst = np.asarray(edge_index[1], dtype=np.int64)

    # in-degree incl. self loop -> symmetric normalization factors
    deg = (np.bincount(dst, minlength=n) + 1).astype(np.float32)
    dinv = (1.0 / np.sqrt(deg)).astype(np.float32)

    loops = np.arange(n, dtype=np.int64)
    src_a = np.concatenate([src, loops])
    dst_a = np.concatenate([dst, loops])
    norm_a = (dinv[src_a] * dinv[dst_a]).astype(np.float32)

    tiles_total = -(-n // tile_dst)
    tiles_total = -(-tiles_total // n_cores) * n_cores
    tpc = tiles_total // n_cores
    n_pad = tiles_total * tile_dst
    rows_pc = tpc * tile_dst

    tile_of = dst_a // tile_dst
    order = np.lexsort((src_a, tile_of))
    src_s = src_a[order]
    dstl_s = (dst_a[order] - tile_of[order] * tile_dst).astype(np.float32)
    norm_s = norm_a[order]
    tile_s = tile_of[order]

    half = HALF if n_pad > HALF else n_pad

    counts = np.bincount(tile_s, minlength=tiles_total)
    tile_starts = np.zeros(tiles_total + 1, np.int64)
    tile_starts[1:] = np.cumsum(counts)
    # per (core, tile): lo/hi split position (edges sorted by src)
    lo_counts = np.zeros(tiles_total, np.int64)
    for g in range(tiles_total):
        s0, c = tile_starts[g], counts[g]
        lo_counts[g] = np.searchsorted(src_s[s0 : s0 + c], half)
    hi_counts = counts - lo_counts

    def nblk(c):
        return -(-c // BLK)

    NB_lo = np.zeros(tpc, np.int64)
    NB_hi = np.zeros(tpc, np.int64)
    for i in range(tpc):
        g = np.arange(n_cores) * tpc + i
        NB_lo[i] = nblk(lo_counts[g]).max()
        NB_hi[i] = nblk(hi_counts[g]).max()
        if NB_lo[i] + NB_hi[i] == 0:
            NB_lo[i] = 1
    F = NB_lo + NB_hi  # blocks per tile slot
    B = int(F.sum())
    off = np.zeros(tpc, np.int64)
    off[1:] = np.cumsum(F)[:-1]

    idx_flat = np.zeros((n_cores, B * BLK), np.int32)  # per-edge gather index
    dst_t = np.zeros((n_cores, BLK, B), np.float32)
    nrm_t = np.zeros((n_cores, BLK, B), np.float32)
    for k in range(n_cores):
        for i in range(tpc):
            g = k * tpc + i
            s0 = int(tile_starts[g])
            clo, chi = int(lo_counts[g]), int(hi_counts[g])
            o = int(off[i])
            # lo edges -> blocks [o, o+NB_lo), hi -> [o+NB_lo, o+F)
            for (cnt, base_blk, idx_shift, pos) in (
                (clo, o, 0, s0),
                (chi, o + int(NB_lo[i]), half, s0 + clo),
            ):
                if cnt == 0:
                    continue
                nb = nblk(cnt)
                cap = nb * BLK
                bi = np.zeros(cap, np.int32)
                bd = np.zeros(cap, np.float32)
                bn = np.zeros(cap, np.float32)
                bi[:cnt] = src_s[pos : pos + cnt] - idx_shift
                bd[:cnt] = dstl_s[pos : pos + cnt]
                bn[:cnt] = norm_s[pos : pos + cnt]
                e0 = base_blk * BLK
                idx_flat[k][e0 : e0 + cap] = bi
                cols = slice(base_blk, base_blk + nb)
                dst_t[k][:, cols] = bd.reshape(nb, BLK).T
                nrm_t[k][:, cols] = bn.reshape(nb, BLK).T

    # dma_gather int16 index tensor: within a call (<= CALL_BLKS blocks),
    # flat edge i of the call sits at [i % 16, w0 + i // 16], replicated
    # across the eight 16-partition groups. Because calls are aligned to
    # block boundaries and a block is 128 = 8*16 edges, the global wrap
    # below produces exactly the per-call layout for any block range.
    ix16 = np.zeros((n_cores, BLK, B * (BLK // 16)), np.int16)
    for k in range(n_cores):
        v = idx_flat[k].astype(np.int16).reshape(B * (BLK // 16), 16).T
        for c in range(8):
            ix16[k][16 * c : 16 * (c + 1), :] = v

    # fused weights
    Wc64 = np.asarray(Wc, np.float64)
    Wt64 = np.asarray(Wt, np.float64)
    W064 = np.asarray(W0, np.float64)
    bc64 = np.asarray(bc, np.float64)
    B1 = np.eye(d) + Wt64
    C = (Wc64 @ B1).astype(np.float32)
    B2 = (W064 - Wt64).astype(np.float32)
    bp = (bc64 @ B1).astype(np.float32)

    x_pad = np.zeros((n_pad, d), np.float32)
    x_pad[:n] = np.asarray(x, np.float32)

    consts = {
        "cw": C,
        "b2w": B2,
        "bpb": np.broadcast_to(bp, (tile_dst, d)).copy(),
        "iota": np.broadcast_to(
            np.arange(tile_dst, dtype=np.float32), (BLK, tile_dst)
        ).copy(),
        "ident": np.eye(BLK, dtype=np.float32),
    }
    x_hi_arr = x_pad[half:] if n_pad > half else np.zeros((1, d), np.float32)
    in_maps = []
    for k in range(n_cores):
        m = dict(consts)
        m["x_full"] = x_pad[:half]
        m["x_hi"] = x_hi_arr
        m["x_own"] = x_pad[k * rows_pc : (k + 1) * rows_pc].copy()
        m["ix16"] = ix16[k]
        m["dst_t"] = dst_t[k]
        m["nrm_t"] = nrm_t[k]
        in_maps.append(m)

    meta = dict(
        F=F,
        NB_lo=NB_lo,
        NB_hi=NB_hi,
        off=off,
        B=B,
        tpc=tpc,
        n_pad=n_pad,
        rows_pc=rows_pc,
        d=d,
        half=half,
        hi_rows=x_hi_arr.shape[0],
    )
    return in_maps, meta


def _build_nc(meta, n_cores=N_CORES, tile_dst=TILE_DST, repeat=1):
    import contextlib

    import concourse.bacc as bacc
    import concourse.mybir as mybir
    import concourse.tile as tile
    from concourse import library_config

    f32 = mybir.dt.float32
    i16 = mybir.dt.int16
    F, NB_lo, NB_hi, off = meta["F"], meta["NB_lo"], meta["NB_hi"], meta["off"]
    B, tpc = meta["B"], meta["tpc"]
    n_pad, rows_pc, d = meta["n_pad"], meta["rows_pc"], meta["d"]
    W16 = B * (BLK // 16)

    nc = bacc.Bacc("TRN2", target_bir_lowering=False, debug=False, num_devices=n_cores)
    x_full = nc.declare_dram_parameter("x_full", [meta["half"], d], f32, isOutput=False)
    x_hi = nc.declare_dram_parameter("x_hi", [meta["hi_rows"], d], f32, isOutput=False)
    x_own = nc.declare_dram_parameter("x_own", [rows_pc, d], f32, isOutput=False)
    ix16 = nc.declare_dram_parameter("ix16", [BLK, W16], i16, isOutput=False)
    dst_t = nc.declare_dram_parameter("dst_t", [BLK, B], f32, isOutput=False)
    nrm_t = nc.declare_dram_parameter("nrm_t", [BLK, B], f32, isOutput=False)
    cw = nc.declare_dram_parameter("cw", [d, d], f32, isOutput=False)
    b2w = nc.declare_dram_parameter("b2w", [d, d], f32, isOutput=False)
    bpb = nc.declare_dram_parameter("bpb", [tile_dst, d], f32, isOutput=False)
    iota = nc.declare_dram_parameter("iota", [BLK, tile_dst], f32, isOutput=False)
    ident = nc.declare_dram_parameter("ident", [BLK, BLK], f32, isOutput=False)
    out = nc.declare_dram_parameter("out", [rows_pc, d], f32, isOutput=True)

    eq, mul, add = (
        mybir.AluOpType.is_equal,
        mybir.AluOpType.mult,
        mybir.AluOpType.add,
    )

    with tile.TileContext(nc) as tc:
        with (
            tc.tile_pool(name="const", bufs=1) as cpool,
            tc.tile_pool(name="tbl", bufs=1) as tpool,
            tc.tile_pool(name="gather", bufs=3) as gpool,
            tc.tile_pool(name="work", bufs=3) as wpool,
            tc.tile_pool(name="oh", bufs=4) as ohpool,
            tc.tile_pool(name="zps", bufs=2, space="PSUM") as zpool,
            tc.tile_pool(name="tps", bufs=2, space="PSUM") as tpspool,
            tc.tile_pool(name="ops", bufs=2, space="PSUM") as opool,
        ):
            nc.gpsimd.load_library(library_config.mlp)
            c_sb = cpool.tile([d, d], f32)
            nc.sync.dma_start(out=c_sb[:], in_=cw[:])
            b2_sb = cpool.tile([d, d], f32)
            nc.sync.dma_start(out=b2_sb[:], in_=b2w[:])
            bp_sb = cpool.tile([tile_dst, d], f32)
            nc.sync.dma_start(out=bp_sb[:], in_=bpb[:])
            io_sb = cpool.tile([BLK, tile_dst], f32)
            nc.sync.dma_start(out=io_sb[:], in_=iota[:])
            id_sb = cpool.tile([BLK, BLK], f32)
            nc.sync.dma_start(out=id_sb[:], in_=ident[:])
            ix_sb = tpool.tile([BLK, W16], i16)
            nc.sync.dma_start(out=ix_sb[:], in_=ix16[:])
            dl_sb = tpool.tile([BLK, B], f32)
            nc.sync.dma_start(out=dl_sb[:], in_=dst_t[:])
            nm_sb = tpool.tile([BLK, B], f32)
            nc.sync.dma_start(out=nm_sb[:], in_=nrm_t[:])

            # repeat>1 wraps the whole body in a device-side loop; used only
            # by the timing harness to amplify device time vs host overhead.
            rep_ctx = tc.For_i(0, repeat, 1) if repeat > 1 else contextlib.nullcontext()
            with rep_ctx:
                for i in range(tpc):
                    fi = int(F[i])
                    o = int(off[i])
                    g = gpool.tile([BLK, fi * d], f32, tag="g")
                    for (tbl, blk0, nb_total) in (
                        (x_full, 0, int(NB_lo[i])),
                        (x_hi, int(NB_lo[i]), int(NB_hi[i])),
                    ):
                        for c in range(0, nb_total, CALL_BLKS):
                            nb = min(CALL_BLKS, nb_total - c)
                            col = blk0 + c
                            nidx = nb * BLK
                            nc.gpsimd.dma_gather(
                                out_ap=g[:, col * d : (col + nb) * d].rearrange(
                                    "p (n e) -> p n e", e=d
                                ),
                                in_ap=tbl[:],
                                idxs_ap=ix_sb[:, (o + col) * 8 : (o + col + nb) * 8],
                                num_idxs=nidx,
                                num_idxs_reg=nidx,
                                elem_size=d,
                            )
                    xo = wpool.tile([tile_dst, d], f32, tag="xo")
                    nc.sync.dma_start(
                        out=xo[:], in_=x_own[i * tile_dst : (i + 1) * tile_dst, :]
                    )
                    xt_ps = tpspool.tile([d, tile_dst], f32)
                    nc.tensor.transpose(
                        out=xt_ps[:], in_=xo[:], identity=id_sb[:tile_dst, :tile_dst]
                    )
                    xt_sb = wpool.tile([d, tile_dst], f32, tag="xt")
                    nc.scalar.copy(out=xt_sb[:], in_=xt_ps[:])

                    z_ps = zpool.tile([d, tile_dst], f32)
                    for j in range(fi):
                        oh = ohpool.tile([BLK, tile_dst], f32, tag="oh")
                        nc.vector.tensor_scalar(
                            out=oh[:],
                            in0=io_sb[:],
                            scalar1=dl_sb[:, o + j : o + j + 1],
                            scalar2=nm_sb[:, o + j : o + j + 1],
                            op0=eq,
                            op1=mul,
                        )
                        nc.tensor.matmul(
                            out=z_ps[:],
                            lhsT=g[:, j * d : (j + 1) * d],
                            rhs=oh[:],
                            start=(j == 0),
                            stop=(j == fi - 1),
                        )
                    z_sb = wpool.tile([d, tile_dst], f32, tag="z")
                    nc.scalar.copy(out=z_sb[:], in_=z_ps[:])

                    o_ps = opool.tile([tile_dst, d], f32)
                    nc.tensor.matmul(
                        out=o_ps[:], lhsT=z_sb[:], rhs=c_sb[:], start=True, stop=False
                    )
                    nc.tensor.matmul(
                        out=o_ps[:], lhsT=xt_sb[:], rhs=b2_sb[:], start=False, stop=True
                    )
                    o_sb = wpool.tile([tile_dst, d], f32, tag="o")
                    nc.vector.tensor_tensor(
                        out=o_sb[:], in0=o_ps[:], in1=bp_sb[:], op=add
                    )
                    nc.sync.dma_start(
                        out=out[i * tile_dst : (i + 1) * tile_dst, :], in_=o_sb[:]
                    )
    nc.compile()
    return nc


def _get_nc(meta, n_cores=N_CORES, tile_dst=TILE_DST):
    key = (tuple(int(f) for f in meta["F"]), tuple(int(f) for f in meta["NB_lo"]))
    if key not in _NC_CACHE:
        _NC_CACHE[key] = _build_nc(meta, n_cores=n_cores, tile_dst=tile_dst)
    return _NC_CACHE[key]


_LAST_RESULTS = None


def kernel(x, edge_index, Wc, bc, W0, Wt):
    global _LAST_RESULTS
    from concourse.bass_utils import run_bass_kernel_spmd

    x = np.asarray(x)
    n = x.shape[0]
    in_maps, meta = _host_prep(x, edge_index, Wc, bc, W0, Wt)
    nc = _get_nc(meta)
    res = run_bass_kernel_spmd(nc, in_maps, list(range(N_CORES)))
    _LAST_RESULTS = res
    outs = [res.results[k]["out"] for k in range(N_CORES)]
    return np.concatenate(outs, axis=0)[:n].astype(np.float32)
